# revision 28
# baseline (speedup 1.0000x reference)
"""Trainium2 Bass kernel for nn_DMS_STAttention_48722109006179.

Mathematical note (load-bearing): in the reference, `_attention_layer`
ends with softmax over axis=-1, which is the head dimension of size
H=1.  Softmax over a single-element axis is identically 1.0, so the
entire attention computation (linears, edge gather/scatter, LeakyReLU)
collapses and the outputs are exactly

    sa[b, t, i, j, 0] = 1.0 + sa_bias[t, i, j]
    ta[b, j, s, t, 0] = 1.0 + ta_bias[j, s, t]

independent of `src` and all weights (verified bit-exact against the
jax reference on device).  The kernel is therefore a pure memory-regime
problem: materialize ~58 MB of broadcast output.  Sharding: pure data
parallel over batch B=2048 across 8 cores (256 batch rows per core).

Per-core device program (raw bass — the toolchain here encodes at most
ONE semaphore wait per instruction, which rules out TileContext's
auto-drain; explicit standalone wait_ge instructions are used instead):

  1. One DMA loads the bias rows in an exact bf16x3 decomposition
     (hi/mid/lo split the fp32 mantissa into disjoint 8-bit chunks, so
     hi+mid+lo reconstructs the fp32 value exactly; the host prepares
     and asserts this).  Layout [3, 128(ones) + 4840(sa) + 2200(ta)].
  2. The TensorEngine broadcasts each column chunk across the 128
     partitions with a K=3 all-ones bf16 matmul into fp32 PSUM —
     one single-pass bf16 matmul instead of the PE's 4-pass internal
     fp32 mode (~2x faster chunk cadence), still bit-exact.
  3. The DVE fuses the +1.0 into the PSUM->SBUF copy (fp32).
  4. Finished column groups stream to the DRAM outputs per _WPLAN:
     row-block 0 on the SP HWDGE queue, row-block 1 on the ACT queue,
     the ta tile written last on both.  The DMA rings carry nothing
     but output writes, so the write stream runs at the HBM cap from
     the first chunk on.
"""

import sys
import types

import ml_dtypes
import numpy as np

import concourse.bass as bass
from concourse import mybir
from concourse.bass_utils import run_bass_kernel_spmd


def _ensure_axon_hooks_shim():
    """bass_utils imports antenv.axon_hooks when tracing; this image's
    antenv package lacks it.  Provide a working shim (ctypes NTFF
    profiling via the axon .so) so trace=True works and a BASS_TRACE=1
    environment doesn't crash the plain run."""
    try:
        import antenv
    except ImportError:
        return
    if "antenv.axon_hooks" in sys.modules:
        return
    mod = types.ModuleType("antenv.axon_hooks")
    try:
        from trn_agent_boot.trn_boot import _ntff_profile_via_ctypes

        mod._hook = _ntff_profile_via_ctypes("/opt/axon/libaxon_pjrt.so")
    except Exception:
        mod._hook = None
    mod.get_axon_ntff_profile_hook = lambda: mod._hook
    mod.set_axon_ntff_profile_hook = lambda h: setattr(mod, "_hook", h)
    sys.modules["antenv.axon_hooks"] = mod
    antenv.axon_hooks = mod


_ensure_axon_hooks_shim()

N_CORES = 8
B = 2048
T = 10
J = 22
SA = T * J * J  # 4840
TA = J * T * T  # 2200
BPC = B // N_CORES  # 256 batch rows per core

_SA_CHUNK = 484  # 10 sa chunks (psum: 484*4 B inside a 2 KB bank)
_TA_CHUNK = 440  # 5 ta chunks
# chunk table: (is_sa, dst col, src col in bias3, width)
_CHUNKS = [
    (True, c, 128 + c, _SA_CHUNK) for c in range(0, SA, _SA_CHUNK)
] + [
    (False, c, 128 + SA + c, _TA_CHUNK) for c in range(0, TA, _TA_CHUNK)
]
_N_CH = len(_CHUNKS)  # 15

# sa write plan per row block: (s_cp gate, col start, col end).  First
# two units are single chunks so the write stream starts as early as
# possible; the rest are double-chunk units.  The ta tile (gate 15)
# is written after these.
_WPLAN = [
    (1, 0, 484),
    (2, 484, 968),
    (4, 968, 1936),
    (6, 1936, 2904),
    (8, 2904, 3872),
    (10, 3872, 4840),
]

# test.py hooks (ignored by the grading harness)
TRACE = False
LAST_EXEC_NS = None

_NC_CACHE = {}


def _build_nc():
    nc = bass.Bass()
    f32 = mybir.dt.float32
    bf16 = mybir.dt.bfloat16

    # rows: hi / mid / lo bf16 planes; cols: [ones(128)|sa(4840)|ta(2200)]
    bias3 = nc.dram_tensor(
        "bias3", [3, 128 + SA + TA], bf16, kind="ExternalInput"
    )
    out_sa = nc.dram_tensor("out_sa", [BPC, SA], f32, kind="ExternalOutput")
    out_ta = nc.dram_tensor("out_ta", [BPC, TA], f32, kind="ExternalOutput")

    with (
        nc.semaphore("s_in0") as s_in0,      # ones + chunk0 planes landed
        nc.semaphore("s_in1") as s_in1,      # rest of bias planes landed
        nc.semaphore("s_mm") as s_mm,        # matmuls done (1 each)
        nc.semaphore("s_cp") as s_cp,        # copies done (1 each)
        nc.semaphore("s_wsp") as s_wsp,      # SP-queue write completions
        nc.semaphore("s_wact") as s_wact,    # ACT-queue write completions
        nc.sbuf_tensor("brow3", [3, 128 + SA + TA], bf16) as brow3,
        nc.sbuf_tensor("t_sa", [128, SA], f32) as t_sa,
        nc.sbuf_tensor("t_ta", [128, TA], f32) as t_ta,
        nc.psum_tensor("acc", [128, 4096], f32) as acc,
    ):
        with nc.Block() as block:

            _D0 = 128 + _SA_CHUNK  # ones + sa chunk 0

            @block.sync
            def _(sync):
                # tiny first load (same queue, FIFO) lets the PE start
                # ~1.5us earlier on chunk 0
                sync.dma_start(
                    out=brow3[:, 0:_D0], in_=bias3[:, 0:_D0]
                ).then_inc(s_in0, 16)
                sync.dma_start(
                    out=brow3[:, _D0:], in_=bias3[:, _D0:]
                ).then_inc(s_in1, 16)
                # row-block 0 sa writes stream behind the copies
                for gate, c0, c1 in _WPLAN:
                    sync.wait_ge(s_cp, gate)
                    sync.dma_start(
                        out=out_sa[0:128, c0:c1], in_=t_sa[:, c0:c1]
                    ).then_inc(s_wsp, 16)
                sync.wait_ge(s_cp, _N_CH)
                sync.dma_start(
                    out=out_ta[128:256, :], in_=t_ta[:]
                ).then_inc(s_wsp, 16)
                sync.wait_ge(s_wsp, 16 * (len(_WPLAN) + 1))

            @block.scalar
            def _(scalar):
                # row-block 1 sa writes + ta row-block 0
                for gate, c0, c1 in _WPLAN:
                    scalar.wait_ge(s_cp, gate)
                    scalar.dma_start(
                        out=out_sa[128:256, c0:c1], in_=t_sa[:, c0:c1]
                    ).then_inc(s_wact, 16)
                scalar.wait_ge(s_cp, _N_CH)
                scalar.dma_start(
                    out=out_ta[0:128, :], in_=t_ta[:]
                ).then_inc(s_wact, 16)
                scalar.wait_ge(s_wact, 16 * (len(_WPLAN) + 1))

            @block.tensor
            def _(tensor):
                tensor.wait_ge(s_in0, 16)
                for i, (is_sa, dc, sc, w) in enumerate(_CHUNKS):
                    bank = (i % 8) * 512
                    if i == 1:
                        tensor.wait_ge(s_in1, 16)
                    elif i >= 8:
                        # PSUM bank reuse: consumer copy must be done
                        tensor.wait_ge(s_cp, i - 7)
                    # out[m,n] = sum_k ones[k,m] * brow3[k, sc+n], K=3:
                    # single-pass bf16 matmul reconstructs fp32 exactly
                    tensor.matmul(
                        acc[:, bank : bank + w],
                        brow3[0:3, 0:128],
                        brow3[0:3, sc : sc + w],
                    ).then_inc(s_mm)

            @block.vector
            def _(vector):
                for i, (is_sa, dc, sc, w) in enumerate(_CHUNKS):
                    bank = (i % 8) * 512
                    dst = t_sa if is_sa else t_ta
                    vector.wait_ge(s_mm, i + 1)
                    vector.tensor_scalar_add(
                        dst[:, dc : dc + w], acc[:, bank : bank + w], 1.0
                    ).then_inc(s_cp)

    return nc


def _get_nc():
    if "nc" not in _NC_CACHE:
        _NC_CACHE["nc"] = _build_nc()
    return _NC_CACHE["nc"]


def _bias3_planes(sa_bias, ta_bias):
    """Exact bf16x3 split: row = hi + mid + lo with disjoint 8-bit
    mantissa chunks, so the device-side fp32 sum is bit-exact."""
    row = np.concatenate(
        [np.ones(128, np.float32), sa_bias.ravel(), ta_bias.ravel()]
    )
    bf = ml_dtypes.bfloat16
    hi = row.astype(bf)
    r1 = row - hi.astype(np.float32)
    mid = r1.astype(bf)
    lo = (r1 - mid.astype(np.float32)).astype(bf)
    rec = hi.astype(np.float32) + mid.astype(np.float32) + lo.astype(np.float32)
    assert np.array_equal(rec, row), "bf16x3 decomposition not exact"
    planes = np.stack([hi, mid, lo])  # [3, 7168] bf16
    planes[:, 0:128] = np.float32(1.0)  # all-ones stationary, all planes
    return planes


def kernel(**inputs):
    global LAST_EXEC_NS
    sa_bias = np.ascontiguousarray(inputs["sa_bias"], dtype=np.float32)
    ta_bias = np.ascontiguousarray(inputs["ta_bias"], dtype=np.float32)
    bias3 = _bias3_planes(sa_bias, ta_bias)

    nc = _get_nc()
    in_maps = [{"bias3": bias3} for _ in range(N_CORES)]
    res = run_bass_kernel_spmd(nc, in_maps, list(range(N_CORES)), trace=TRACE)
    LAST_EXEC_NS = res.exec_time_ns

    sa = np.concatenate(
        [r["out_sa"].reshape(BPC, T, J, J, 1) for r in res.results], axis=0
    )
    ta = np.concatenate(
        [r["out_ta"].reshape(BPC, J, T, T, 1) for r in res.results], axis=0
    )
    return sa, ta


# revision 29
# speedup vs baseline: 1.1118x; 1.1118x over previous
"""Trainium2 Bass kernel for nn_DMS_STAttention_48722109006179.

Mathematical note (load-bearing): in the reference, `_attention_layer`
ends with softmax over axis=-1, which is the head dimension of size
H=1.  Softmax over a single-element axis is identically 1.0, so the
entire attention computation (linears, edge gather/scatter, LeakyReLU)
collapses and the outputs are exactly

    sa[b, t, i, j, 0] = 1.0 + sa_bias[t, i, j]
    ta[b, j, s, t, 0] = 1.0 + ta_bias[j, s, t]

independent of `src` and all weights (verified bit-exact against the
jax reference on device).  The kernel is therefore a pure memory-regime
problem: materialize ~58 MB of broadcast output.  Sharding: pure data
parallel over batch B=2048 across 8 cores (256 batch rows per core).

Per-core device program (raw bass — the toolchain here encodes at most
ONE semaphore wait per instruction, which rules out TileContext's
auto-drain; explicit standalone wait_ge instructions are used instead):

  1. One DMA loads the bias rows in an exact bf16x3 decomposition
     (hi/mid/lo split the fp32 mantissa into disjoint 8-bit chunks, so
     hi+mid+lo reconstructs the fp32 value exactly; the host prepares
     and asserts this).  Layout [3, 128(ones) + 4840(sa) + 2200(ta)].
  2. The TensorEngine broadcasts each column chunk across the 128
     partitions with a K=3 all-ones bf16 matmul into fp32 PSUM —
     one single-pass bf16 matmul instead of the PE's 4-pass internal
     fp32 mode (~2x faster chunk cadence), still bit-exact.
  3. The DVE fuses the +1.0 into the PSUM->SBUF copy (fp32).
  4. Finished column groups stream to the DRAM outputs per _WPLAN:
     row-block 0 on the SP HWDGE queue, row-block 1 on the ACT queue,
     the ta tile written last on both.  The DMA rings carry nothing
     but output writes, so the write stream runs at the HBM cap from
     the first chunk on.
"""

import sys
import types

import ml_dtypes
import numpy as np

import concourse.bass as bass
from concourse import mybir
from concourse.bass_utils import run_bass_kernel_spmd


def _ensure_axon_hooks_shim():
    """bass_utils imports antenv.axon_hooks when tracing; this image's
    antenv package lacks it.  Provide a working shim (ctypes NTFF
    profiling via the axon .so) so trace=True works and a BASS_TRACE=1
    environment doesn't crash the plain run."""
    try:
        import antenv
    except ImportError:
        return
    if "antenv.axon_hooks" in sys.modules:
        return
    mod = types.ModuleType("antenv.axon_hooks")
    try:
        from trn_agent_boot.trn_boot import _ntff_profile_via_ctypes

        mod._hook = _ntff_profile_via_ctypes("/opt/axon/libaxon_pjrt.so")
    except Exception:
        mod._hook = None
    mod.get_axon_ntff_profile_hook = lambda: mod._hook
    mod.set_axon_ntff_profile_hook = lambda h: setattr(mod, "_hook", h)
    sys.modules["antenv.axon_hooks"] = mod
    antenv.axon_hooks = mod


_ensure_axon_hooks_shim()

N_CORES = 8
B = 2048
T = 10
J = 22
SA = T * J * J  # 4840
TA = J * T * T  # 2200
BPC = B // N_CORES  # 256 batch rows per core

_SA_CHUNK = 484  # 10 sa chunks (psum: 484*4 B inside a 2 KB bank)
_TA_CHUNK = 440  # 5 ta chunks
# chunk table: (is_sa, dst col, src col in bias3, width)
_CHUNKS = [
    (True, c, 128 + c, _SA_CHUNK) for c in range(0, SA, _SA_CHUNK)
] + [
    (False, c, 128 + SA + c, _TA_CHUNK) for c in range(0, TA, _TA_CHUNK)
]
_N_CH = len(_CHUNKS)  # 15

# sa write plan per row block: (s_cp gate, col start, col end).  First
# two units are single chunks so the write stream starts as early as
# possible; the rest are double-chunk units.  The ta tile (gate 15)
# is written after these.
_WPLAN = [
    (1, 0, 484),
    (2, 484, 968),
    (4, 968, 1936),
    (6, 1936, 2904),
    (8, 2904, 3872),
    (10, 3872, 4840),
]

# test.py hooks (ignored by the grading harness)
TRACE = False
LAST_EXEC_NS = None

_NC_CACHE = {}


def _build_nc():
    nc = bass.Bass()
    f32 = mybir.dt.float32
    bf16 = mybir.dt.bfloat16

    # rows: hi / mid / lo bf16 planes; cols: [ones(128)|sa(4840)|ta(2200)]
    bias3 = nc.dram_tensor(
        "bias3", [3, 128 + SA + TA], bf16, kind="ExternalInput"
    )
    out_sa = nc.dram_tensor("out_sa", [BPC, SA], f32, kind="ExternalOutput")
    out_ta = nc.dram_tensor("out_ta", [BPC, TA], f32, kind="ExternalOutput")

    with (
        nc.semaphore("s_in0") as s_in0,      # ones + chunk0 planes landed
        nc.semaphore("s_in1") as s_in1,      # rest of bias planes landed
        nc.semaphore("s_mm") as s_mm,        # matmuls done (1 each)
        nc.semaphore("s_cp") as s_cp,        # copies done (1 each)
        nc.semaphore("s_wsp") as s_wsp,      # SP-queue write completions
        nc.semaphore("s_wact") as s_wact,    # ACT-queue write completions
        nc.sbuf_tensor("brow3", [3, 128 + SA + TA], bf16) as brow3,
        nc.sbuf_tensor("t_sa", [128, SA], f32) as t_sa,
        nc.sbuf_tensor("t_ta", [128, TA], f32) as t_ta,
        nc.psum_tensor("acc", [128, 4096], f32) as acc,
    ):
        with nc.Block() as block:

            _D0 = 128 + 2 * _SA_CHUNK  # ones + sa chunks 0-1

            @block.sync
            def _(sync):
                # tiny first load (same queue, FIFO) lets the PE start
                # ~1.5us earlier on chunk 0
                sync.dma_start(
                    out=brow3[:, 0:_D0], in_=bias3[:, 0:_D0]
                ).then_inc(s_in0, 16)
                sync.dma_start(
                    out=brow3[:, _D0:], in_=bias3[:, _D0:]
                ).then_inc(s_in1, 16)
                # row-block 0 sa writes stream behind the copies
                for gate, c0, c1 in _WPLAN:
                    sync.wait_ge(s_cp, gate)
                    sync.dma_start(
                        out=out_sa[0:128, c0:c1], in_=t_sa[:, c0:c1]
                    ).then_inc(s_wsp, 16)
                sync.wait_ge(s_cp, _N_CH)
                sync.dma_start(
                    out=out_ta[128:256, :], in_=t_ta[:]
                ).then_inc(s_wsp, 16)
                sync.wait_ge(s_wsp, 16 * (len(_WPLAN) + 1))

            @block.scalar
            def _(scalar):
                # row-block 1 sa writes + ta row-block 0
                for gate, c0, c1 in _WPLAN:
                    scalar.wait_ge(s_cp, gate)
                    scalar.dma_start(
                        out=out_sa[128:256, c0:c1], in_=t_sa[:, c0:c1]
                    ).then_inc(s_wact, 16)
                scalar.wait_ge(s_cp, _N_CH)
                scalar.dma_start(
                    out=out_ta[0:128, :], in_=t_ta[:]
                ).then_inc(s_wact, 16)
                scalar.wait_ge(s_wact, 16 * (len(_WPLAN) + 1))

            @block.tensor
            def _(tensor):
                tensor.wait_ge(s_in0, 16)
                for i, (is_sa, dc, sc, w) in enumerate(_CHUNKS):
                    bank = (i % 8) * 512
                    if i == 2:
                        tensor.wait_ge(s_in1, 16)
                    elif i >= 8:
                        # PSUM bank reuse: consumer copy must be done
                        tensor.wait_ge(s_cp, i - 7)
                    # out[m,n] = sum_k ones[k,m] * brow3[k, sc+n], K=3:
                    # single-pass bf16 matmul reconstructs fp32 exactly
                    tensor.matmul(
                        acc[:, bank : bank + w],
                        brow3[0:3, 0:128],
                        brow3[0:3, sc : sc + w],
                    ).then_inc(s_mm)

            @block.vector
            def _(vector):
                for i, (is_sa, dc, sc, w) in enumerate(_CHUNKS):
                    bank = (i % 8) * 512
                    dst = t_sa if is_sa else t_ta
                    vector.wait_ge(s_mm, i + 1)
                    vector.tensor_scalar_add(
                        dst[:, dc : dc + w], acc[:, bank : bank + w], 1.0
                    ).then_inc(s_cp)

    return nc


def _get_nc():
    if "nc" not in _NC_CACHE:
        _NC_CACHE["nc"] = _build_nc()
    return _NC_CACHE["nc"]


def _bias3_planes(sa_bias, ta_bias):
    """Exact bf16x3 split: row = hi + mid + lo with disjoint 8-bit
    mantissa chunks, so the device-side fp32 sum is bit-exact."""
    row = np.concatenate(
        [np.ones(128, np.float32), sa_bias.ravel(), ta_bias.ravel()]
    )
    bf = ml_dtypes.bfloat16
    hi = row.astype(bf)
    r1 = row - hi.astype(np.float32)
    mid = r1.astype(bf)
    lo = (r1 - mid.astype(np.float32)).astype(bf)
    rec = hi.astype(np.float32) + mid.astype(np.float32) + lo.astype(np.float32)
    assert np.array_equal(rec, row), "bf16x3 decomposition not exact"
    planes = np.stack([hi, mid, lo])  # [3, 7168] bf16
    planes[:, 0:128] = np.float32(1.0)  # all-ones stationary, all planes
    return planes


def kernel(**inputs):
    global LAST_EXEC_NS
    sa_bias = np.ascontiguousarray(inputs["sa_bias"], dtype=np.float32)
    ta_bias = np.ascontiguousarray(inputs["ta_bias"], dtype=np.float32)
    bias3 = _bias3_planes(sa_bias, ta_bias)

    nc = _get_nc()
    in_maps = [{"bias3": bias3} for _ in range(N_CORES)]
    res = run_bass_kernel_spmd(nc, in_maps, list(range(N_CORES)), trace=TRACE)
    LAST_EXEC_NS = res.exec_time_ns

    sa = np.concatenate(
        [r["out_sa"].reshape(BPC, T, J, J, 1) for r in res.results], axis=0
    )
    ta = np.concatenate(
        [r["out_ta"].reshape(BPC, J, T, T, 1) for r in res.results], axis=0
    )
    return sa, ta
